# revision 5
# baseline (speedup 1.0000x reference)
"""Trainium2 Bass kernel for batched scaled-dot-product attention.

Problem (all fp32):
    q = queries @ Wq + bq          [B=4, N=4096, E=64]   (D_MODEL=768)
    k = keys    @ Wk + bk
    v = values  @ Wv + bv
    out = softmax(q k^T / sqrt(E)) @ v                    [B, N, 64]

Sharding: 8 cores, data-parallel over batch x query-half.  Core c handles
batch b=c//2, query rows [h*2048, (h+1)*2048) with h=c%2; it loads the full
keys/values for its batch (softmax needs every key).

Per-core algorithm (all matmuls fp32r = full-rate fp32-reduced):
  1. Transpose inputs 128x128-blockwise on the PE (contraction dim 768 must
     sit on partitions), project with W as the stationary operand.  This
     yields qT/kT [64, seq] directly (scores need the E dim on partitions).
     The q bias/scale are folded into the PSUM->SBUF copy on the ACT engine.
  2. v is projected to vT [64, 4096] and PE-transposed back to natural
     [4096, 64], augmented with a ones column (row sums of unnormalized
     attention weights fall out of the attn @ v_aug matmul for free).
  3. Attention per (k-tile kt, q-group g): S^T tile [128k, 512q] = kT_kt^T qT_g;
     P^T = exp(S^T) on ACT (no max subtraction: scores ~ N(0,1), exp is safe);
     oT[g] [65, 512] += v_aug_kt^T P^T accumulated over kt in PSUM.
     The S^T layout avoids transposing the 4096-wide attention matrix.
  4. Epilogue: PE-transpose oT to natural [512, 65]; divide the 64 value
     columns by the ones-column row sum; DMA out.
"""

import numpy as np

B, N, D, E = 4, 4096, 768, 64
NCORES = 8
HALF = N // 2          # query rows per core
CH = D // 128          # 6 feature chunks of the contraction dim
GT = 4                 # seq tiles per projection group (512-wide moving dim)
GROUP = 128 * GT       # 512
KT = N // 128          # 32 key tiles
QG = HALF // GROUP     # 4 query groups per core
SCALE = 1.0 / 8.0      # 1/sqrt(E)

_CACHE = {}


def _build():
    from contextlib import ExitStack

    import concourse.mybir as mybir
    from concourse import bacc
    import concourse.tile as tile
    from concourse.masks import make_identity

    f32 = mybir.dt.float32
    f32r = mybir.dt.float32r
    EXP = mybir.ActivationFunctionType.Exp
    IDENT = mybir.ActivationFunctionType.Identity

    nc = bacc.Bacc(trn_type="TRN2")
    x_q = nc.dram_tensor("x_q", [HALF, D], f32, kind="ExternalInput")
    x_k = nc.dram_tensor("x_k", [N, D], f32, kind="ExternalInput")
    x_v = nc.dram_tensor("x_v", [N, D], f32, kind="ExternalInput")
    w_q = nc.dram_tensor("w_q", [D, E], f32, kind="ExternalInput")
    w_k = nc.dram_tensor("w_k", [D, E], f32, kind="ExternalInput")
    w_v = nc.dram_tensor("w_v", [D, E], f32, kind="ExternalInput")
    b_q = nc.dram_tensor("b_q", [E], f32, kind="ExternalInput")
    b_k = nc.dram_tensor("b_k", [E], f32, kind="ExternalInput")
    b_v = nc.dram_tensor("b_v", [E], f32, kind="ExternalInput")
    out = nc.dram_tensor("out", [HALF, E], f32, kind="ExternalOutput")

    with tile.TileContext(nc) as tc, ExitStack() as ctx:
        singles = ctx.enter_context(tc.tile_pool(name="singles", bufs=1))
        ident = singles.tile([128, 128], f32)
        make_identity(nc, ident)

        wq_sb = singles.tile([128, CH, E], f32r)
        wk_sb = singles.tile([128, CH, E], f32r)
        wv_sb = singles.tile([128, CH, E], f32r)
        wstage = singles.tile([128, 3, CH, E], f32)
        for i, w_dr in enumerate((w_q, w_k, w_v)):
            nc.sync.dma_start(
                out=wstage[:, i], in_=w_dr.rearrange("(c p) e -> p c e", p=128))
        for i, w_sb in enumerate((wq_sb, wk_sb, wv_sb)):
            nc.vector.tensor_copy(w_sb, wstage[:, i])
        bq_sb = singles.tile([E, 1], f32)
        bk_sb = singles.tile([E, 1], f32)
        bv_sb = singles.tile([E, 1], f32)
        for b_sb, b_dr in ((bq_sb, b_q), (bk_sb, b_k), (bv_sb, b_v)):
            nc.sync.dma_start(out=b_sb, in_=b_dr.rearrange("(p one) -> p one", one=1))
        bqs_sb = singles.tile([E, 1], f32)
        nc.scalar.mul(bqs_sb, bq_sb, SCALE)  # q bias pre-scaled by 1/sqrt(E)

        qT = singles.tile([E, HALF], f32r)      # scaled q^T
        kT = singles.tile([E, N], f32r)
        vT = singles.tile([E, N], f32)
        va = singles.tile([128, KT, E + 1], f32r)  # v natural + ones column
        ones_sb = singles.tile([128, KT], f32)
        nc.vector.memset(ones_sb, 1.0)
        nc.vector.tensor_copy(va[:, :, E], ones_sb)

        with ExitStack() as pro:
            xn_pool = pro.enter_context(tc.tile_pool(name="xn", bufs=3))
            xT_pool = pro.enter_context(tc.tile_pool(name="xT", bufs=2))
            tp_psum = pro.enter_context(tc.tile_pool(name="tp", bufs=2, space="PSUM"))
            pj_psum = pro.enter_context(tc.tile_pool(name="pj", bufs=2, space="PSUM"))
            vt_psum = pro.enter_context(tc.tile_pool(name="vt", bufs=2, space="PSUM"))

            def project(x_dr, n_rows, w_sb, bias, dst, scale):
                for g in range(n_rows // GROUP):
                    xn = xn_pool.tile([128, GT, D], f32, tag="xn")
                    nc.sync.dma_start(
                        out=xn,
                        in_=x_dr[g * GROUP:(g + 1) * GROUP, :].rearrange(
                            "(t p) d -> p t d", p=128),
                    )
                    xT = xT_pool.tile([128, GT, CH, 128], f32r, tag="xT")
                    for c in range(CH):
                        pt = tp_psum.tile([128, GT, 128], f32, tag="tp")
                        for t in range(GT):
                            nc.tensor.transpose(
                                pt[:, t, :], xn[:, t, c * 128:(c + 1) * 128], ident)
                        nc.vector.tensor_copy(xT[:, :, c, :], pt)
                    ps = pj_psum.tile([E, GROUP], f32, tag="pj")
                    for c in range(CH):
                        nc.tensor.matmul(
                            ps,
                            lhsT=w_sb[:, c, :],
                            rhs=xT[:, :, c, :],
                            start=(c == 0),
                            stop=(c == CH - 1),
                        )
                    nc.scalar.activation(
                        dst[:, g * GROUP:(g + 1) * GROUP], ps, IDENT,
                        bias=bias, scale=scale)
                    yield g

            for _ in project(x_q, HALF, wq_sb, bqs_sb, qT, SCALE):
                pass
            for _ in project(x_k, N, wk_sb, bk_sb, kT, 1.0):
                pass
            for g in project(x_v, N, wv_sb, bv_sb, vT, 1.0):
                # v_aug chunks for the 4 k-tiles this group just produced
                for kt in range(GT * g, GT * (g + 1)):
                    po = vt_psum.tile([128, E], f32, tag="vt")
                    nc.tensor.transpose(
                        po, vT[:, kt * 128:(kt + 1) * 128], ident[:E, :E])
                    nc.vector.tensor_copy(va[:, kt, 0:E], po)

        with ExitStack() as att:
            s_psum = att.enter_context(tc.tile_pool(name="s", bufs=3, space="PSUM"))
            o_psum = att.enter_context(tc.tile_pool(name="o", bufs=1, space="PSUM"))
            ep_psum = att.enter_context(tc.tile_pool(name="ep", bufs=1, space="PSUM"))
            pT_pool = att.enter_context(tc.tile_pool(name="pT", bufs=4))
            ep_pool = att.enter_context(tc.tile_pool(name="epo", bufs=2))

            oT = [
                o_psum.tile([E + 1, GROUP], f32, tag=f"oT{g}", name=f"oT{g}")
                for g in range(QG)
            ]
            for kt in range(KT):
                for g in range(QG):
                    s_ps = s_psum.tile([128, GROUP], f32, tag="s")
                    nc.tensor.matmul(
                        s_ps,
                        lhsT=kT[:, kt * 128:(kt + 1) * 128],
                        rhs=qT[:, g * GROUP:(g + 1) * GROUP],
                        start=True, stop=True,
                    )
                    pT = pT_pool.tile([128, GROUP], f32r, tag="pT")
                    nc.scalar.activation(pT, s_ps, EXP)
                    nc.tensor.matmul(
                        oT[g],
                        lhsT=va[:, kt, :],
                        rhs=pT,
                        start=(kt == 0), stop=(kt == KT - 1),
                        skip_group_check=True,
                    )

            for g in range(QG):
                oT_sb = ep_pool.tile([E + 1, GROUP], f32, tag="oT_sb")
                nc.scalar.copy(oT_sb, oT[g])
                for j in range(GT):
                    op = ep_psum.tile([128, E + 1], f32, tag="op")
                    nc.tensor.transpose(
                        op, oT_sb[:, j * 128:(j + 1) * 128], ident[:E + 1, :E + 1])
                    o_sb = ep_pool.tile([128, E + 1], f32, tag="o_sb")
                    nc.vector.tensor_copy(o_sb, op)
                    rec = ep_pool.tile([128, 1], f32, tag="rec")
                    nc.vector.reciprocal(rec, o_sb[:, E:E + 1])
                    o_fin = ep_pool.tile([128, E], f32, tag="o_fin")
                    nc.vector.tensor_scalar_mul(o_fin, o_sb[:, 0:E], rec)
                    r0 = g * GROUP + j * 128
                    nc.sync.dma_start(out=out[r0:r0 + 128, :], in_=o_fin)

    nc.finalize()
    return nc


def get_nc():
    if "nc" not in _CACHE:
        _CACHE["nc"] = _build()
    return _CACHE["nc"]


def make_in_maps(queries, keys, values, Wq, bq, Wk, bk, Wv, bv):
    def f(a):
        return np.ascontiguousarray(np.asarray(a), dtype=np.float32)

    queries, keys, values = f(queries), f(keys), f(values)
    shared = {
        "w_q": f(Wq), "w_k": f(Wk), "w_v": f(Wv),
        "b_q": f(bq), "b_k": f(bk), "b_v": f(bv),
    }
    in_maps = []
    for c in range(NCORES):
        b, h = divmod(c, 2)
        in_maps.append({
            "x_q": queries[b, h * HALF:(h + 1) * HALF, :],
            "x_k": keys[b],
            "x_v": values[b],
            **shared,
        })
    return in_maps


def run(trace=False, **inputs):
    from concourse.bass_utils import run_bass_kernel_spmd

    nc = get_nc()
    in_maps = make_in_maps(**inputs)
    res = run_bass_kernel_spmd(
        nc, in_maps, core_ids=list(range(NCORES)), trace=trace)
    full = np.empty((B, N, E), dtype=np.float32)
    for c in range(NCORES):
        b, h = divmod(c, 2)
        full[b, h * HALF:(h + 1) * HALF, :] = res.results[c]["out"]
    return full, res


def kernel(**inputs):
    full, _ = run(trace=False, **inputs)
    return full
